# revision 1
# baseline (speedup 1.0000x reference)
"""Distributed GPT-2 attention block for 8 TRN2 NeuronCores.

Sharding: core i handles heads {2i, 2i+1} for BOTH batches (tensor-parallel
column split of c_attn). After attention, one 8-core AllToAll converts
head-sharding to token-sharding (512-token slice of the flattened [4096]
token axis per core), then each core runs c_proj (full 1024-feature
contraction) for its slice. Host unshard is pure concatenation.

Layout notes: hidden_states is passed pre-transposed [NX, B*S] (bf16, the
kernel's compute dtype) so q,k stay in [d, t] layout; scores are computed
as S^T = kT.T @ qT; the softmax denominator comes for free from an appended
ones-column in v during the PV matmul; causal masking = block skipping +
affine_select zeroing on diagonal tiles; exp and the 1/sqrt(d) scale are
fused into one ACT pass. All matmuls accumulate in f32 PSUM.
"""

import numpy as np
import ml_dtypes
from contextlib import ExitStack

import concourse.bass as bass
import concourse.bacc as bacc
import concourse.mybir as mybir
import concourse.tile as tile
from concourse.bass_utils import run_bass_kernel_spmd

B, S, NX = 2, 2048, 1024
H, D = 16, 64
HPC = 2              # heads per core
GF = HPC * D         # 128 features per head group
NCORES = 8
SF = B * S           # 4096 flattened tokens
TSL = SF // NCORES   # 512-token output slice per core

F32 = mybir.dt.float32
BF16 = mybir.dt.bfloat16


def build(zero_attn_bias: bool, zero_proj_bias: bool) -> bass.Bass:
    nc = bacc.Bacc(None)

    hst = nc.declare_dram_parameter("hst", [NX, SF], BF16, isOutput=False)
    wqkv = nc.declare_dram_parameter("wqkv", [NX, 3 * GF], BF16, isOutput=False)
    bqkv = nc.declare_dram_parameter("bqkv", [3 * GF, 1], F32, isOutput=False)
    wproj = nc.declare_dram_parameter("wproj", [NX, NX], BF16, isOutput=False)
    bproj = nc.declare_dram_parameter("bproj", [NX, 1], F32, isOutput=False)
    out_ext = nc.declare_dram_parameter("out", [NX, TSL], F32, isOutput=True)

    KT = NX // 128   # 8 k tiles

    with tile.TileContext(nc) as tc, ExitStack() as ctx:
        pool1 = ctx.enter_context(tc.tile_pool(name="persist", bufs=1))
        small = ctx.enter_context(tc.tile_pool(name="small", bufs=2))
        ppool = ctx.enter_context(tc.tile_pool(name="ppool", bufs=2))
        psum = ctx.enter_context(tc.tile_pool(name="psum", bufs=2, space="PSUM"))
        psum_av = ctx.enter_context(tc.tile_pool(name="psum_av", bufs=2, space="PSUM"))
        psum_rb = ctx.enter_context(tc.tile_pool(name="psum_rb", bufs=2, space="PSUM"))
        dram = ctx.enter_context(tc.tile_pool(name="dram", bufs=1, space="DRAM"))

        # ---- load weights and hidden states (bf16, direct, no staging) ------
        # 3D-AP DMAs: [part, ktile, col] <- DRAM[ktile*128 + part, col]
        wqkv_bf = pool1.tile([128, KT, 3 * GF], BF16)
        nc.sync.dma_start(
            wqkv_bf[:], wqkv[:, :].rearrange("(kt p) c -> p kt c", p=128))
        hst_bf = pool1.tile([128, KT, SF], BF16)
        for q in range(4):
            tsl = slice(q * (SF // 4), (q + 1) * (SF // 4))
            for kt in range(KT):
                nc.sync.dma_start(
                    hst_bf[:, kt, tsl], hst[kt * 128:(kt + 1) * 128, tsl])

        # biases as per-partition tiles (q/k: feature-per-partition in qkT layout)
        bqk_t = pool1.tile([128, 2, 1], F32)   # ft 0 = q(2 heads), ft 1 = k
        if not zero_attn_bias:
            for ft in range(2):
                nc.sync.dma_start(bqk_t[:, ft, :], bqkv[ft * 128:(ft + 1) * 128, :])
        bv_t = pool1.tile([64, HPC, 1], F32)
        if not zero_attn_bias:
            for h in range(HPC):
                nc.sync.dma_start(
                    bv_t[:, h, :], bqkv[2 * GF + h * D:2 * GF + (h + 1) * D, :])
        bproj_t = pool1.tile([128, KT, 1], F32)
        if not zero_proj_bias:
            nc.sync.dma_start(
                bproj_t[:], bproj[:, :].rearrange("(kt p) c -> p kt c", p=128))

        # ---- QKV projection --------------------------------------------------
        # q,k transposed: qk_sb[:, ft, t]; ft 0 = q (2 heads), ft 1 = k
        qk_sb = pool1.tile([128, 2, SF], BF16)
        v_sb = pool1.tile([128, SF // 128, HPC, D + 1], BF16)

        def project_batch(b):
            for ft in range(2):
                for tch in range(4 * b, 4 * b + 4):
                    ps = psum.tile([128, 2, 512], F32, tag="mm")
                    for kt in range(KT):
                        nc.tensor.matmul(
                            ps[:, 0, :],
                            lhsT=wqkv_bf[:, kt, ft * 128:(ft + 1) * 128],
                            rhs=hst_bf[:, kt, tch * 512:(tch + 1) * 512],
                            start=(kt == 0), stop=(kt == KT - 1),
                        )
                    if zero_attn_bias:
                        nc.vector.tensor_copy(
                            qk_sb[:, ft, tch * 512:(tch + 1) * 512], ps[:, 0, :])
                    else:
                        nc.scalar.activation(
                            qk_sb[:, ft, tch * 512:(tch + 1) * 512], ps[:, 0, :],
                            mybir.ActivationFunctionType.Identity,
                            bias=bqk_t[:, ft, :])
            # v natural [token, feat] + ones column: v_sb [128, tt, h, 65]
            for tt in range(16 * b, 16 * b + 16):
                ps = psum.tile([128, 2, 512], F32, tag="mm")
                for kt in range(KT):
                    nc.tensor.matmul(
                        ps[:, 0, 0:GF],
                        lhsT=hst_bf[:, kt, tt * 128:(tt + 1) * 128],
                        rhs=wqkv_bf[:, kt, 2 * GF:3 * GF],
                        start=(kt == 0), stop=(kt == KT - 1),
                    )
                nc.vector.tensor_copy(
                    v_sb[:, tt, :, 0:D],
                    ps[:, 0, 0:GF].rearrange("p (h d) -> p h d", h=HPC))
            nc.gpsimd.memset(v_sb[:, 16 * b:16 * b + 16, :, D:D + 1], 1.0)

        # ---- attention -------------------------------------------------------
        # one A2A per head: the h=0 collective overlaps h=1 attention compute
        a2a_in_h = [dram.tile([NCORES, D, TSL], BF16, name=f"a2ain{h}")
                    for h in range(HPC)]
        a2a_out_h = [dram.tile([NCORES, D, TSL], BF16, name=f"a2aout{h}")
                     for h in range(HPC)]

        ones1 = pool1.tile([1, D], BF16)
        nc.gpsimd.memset(ones1[:], 1.0)
        # shifted triangular mask strip: strip[p, x] = 1.0 iff x - 384 >= p
        strip = pool1.tile([128, 896], BF16)
        nc.gpsimd.memset(strip[:], 1.0)
        nc.gpsimd.affine_select(
            out=strip[:], in_=strip[:],
            compare_op=mybir.AluOpType.is_ge, fill=0.0,
            base=-384, pattern=[[1, 896]], channel_multiplier=-1)

        # staging for all heads'/blocks' normalized attention outputs:
        # [64 part, h, slot(=4b+tb), 512] -> one DMA per h to a2a_in
        atall = pool1.tile([D, HPC, NCORES, TSL], BF16)

        pending = []

        def attn_batch(h, b):
            qrow = 64 * h
            if True:
                tok0 = b * S               # batch token offset (flattened)
                tt0 = tok0 // 128          # v tile offset
                for tb in (3, 2, 1, 0):    # big blocks first: short tail chain
                    ntj = 4 * (tb + 1)     # causal: tj tiles 0..ntj-1
                    av = psum_av.tile([D + 1, 512], F32, tag="av")
                    for tjq in range(ntj // 4):   # quads of tj tiles
                        if tjq < ntj // 4 - 1:
                            # full (strictly-lower) quad
                            st_ps = psum.tile([128, 2, 512], F32, tag="mm")
                            pt = ppool.tile([128, 4, 512], BF16, tag="pt")
                            for u in range(4):
                                t = 4 * tjq + u
                                nc.tensor.matmul(
                                    st_ps[:, u % 2, :],
                                    lhsT=qk_sb[qrow:qrow + 64, 1,
                                               tok0 + t * 128:tok0 + (t + 1) * 128],
                                    rhs=qk_sb[qrow:qrow + 64, 0,
                                              tok0 + tb * 512:tok0 + (tb + 1) * 512],
                                    start=True, stop=True,
                                )
                                # exp((q.k)/sqrt(d)); pairs share one psum tile
                                if u % 2 == 1:
                                    nc.scalar.activation(
                                        pt[:, u - 1:u + 1, :], st_ps[:],
                                        mybir.ActivationFunctionType.Exp, scale=0.125)
                                    if u == 1:
                                        st_ps = psum.tile([128, 2, 512], F32, tag="mm")
                            for u in range(4):
                                t = 4 * tjq + u
                                nc.tensor.matmul(
                                    av[:],
                                    lhsT=v_sb[:, tt0 + t, h, :],
                                    rhs=pt[:, u, :],
                                    start=(t == 0), stop=(t == ntj - 1),
                                )
                        else:
                            # diagonal quad: tile u only needs queries
                            # ti >= 128u -> widths 512/384/256/128, packed in
                            # pairs; masked via strip multiply (keep c' >= p)
                            for pair in range(2):
                                st_d = psum.tile([128, 1024], F32, tag="mm")
                                pt_d = ppool.tile([128, 1024], BF16, tag="pt")
                                w0 = 512 - 128 * (2 * pair)
                                w1 = 512 - 128 * (2 * pair + 1)
                                for u2 in range(2):
                                    u = 2 * pair + u2
                                    t = 4 * tjq + u
                                    w = 512 - 128 * u
                                    off = 0 if u2 == 0 else w0
                                    qlo = tok0 + tb * 512 + 128 * u
                                    nc.tensor.matmul(
                                        st_d[:, off:off + w],
                                        lhsT=qk_sb[qrow:qrow + 64, 1,
                                                   tok0 + t * 128:tok0 + (t + 1) * 128],
                                        rhs=qk_sb[qrow:qrow + 64, 0, qlo:qlo + w],
                                        start=True, stop=True,
                                    )
                                nc.scalar.activation(
                                    pt_d[:, 0:w0 + w1], st_d[:, 0:w0 + w1],
                                    mybir.ActivationFunctionType.Exp, scale=0.125)
                                for u2 in range(2):
                                    u = 2 * pair + u2
                                    t = 4 * tjq + u
                                    w = 512 - 128 * u
                                    off = 0 if u2 == 0 else w0
                                    nc.vector.tensor_mul(
                                        pt_d[:, off:off + w], pt_d[:, off:off + w],
                                        strip[:, 384:384 + w])
                                    nc.tensor.matmul(
                                        av[:, 128 * u:512],
                                        lhsT=v_sb[:, tt0 + t, h, :],
                                        rhs=pt_d[:, off:off + w],
                                        start=(t == 0), stop=(t == ntj - 1),
                                    )
                        if pending:
                            pending.pop(0)()
                    # normalize by row D (the P row-sums); recip now (DVE),
                    # but defer the PE broadcast+mul until after the next
                    # block's S quads so PE never waits on DVE here
                    recip = small.tile([1, 512], BF16, tag="recip")
                    with nc.allow_low_precision("softmax recip bf16 is fine"):
                        nc.vector.reciprocal(recip[:], av[D:D + 1, :])

                    def make_epilogue(av=av, recip=recip, h=h, slot=4 * b + tb):
                        def epi():
                            rb = psum_rb.tile([D, 512], F32, tag="rb")
                            nc.tensor.matmul(rb[:], lhsT=ones1[:], rhs=recip[:],
                                             start=True, stop=True)
                            rb_sb = ppool.tile([D, 512], F32, tag="rbs")
                            nc.vector.tensor_copy(rb_sb[:], rb[:])
                            dst = atall[:, h, slot, :]
                            if zero_attn_bias:
                                nc.vector.tensor_mul(dst, av[0:D, :], rb_sb[:])
                            else:
                                at = ppool.tile([D, 512], BF16, tag="at")
                                nc.vector.tensor_mul(at[:], av[0:D, :], rb_sb[:])
                                nc.scalar.activation(
                                    dst, at[:],
                                    mybir.ActivationFunctionType.Identity,
                                    bias=bv_t[:, h, :])
                        return epi
                    pending.append(make_epilogue())

        def head_a2a(h):
            while pending:
                pending.pop(0)()
            # per-head store + AllToAll: [64, slot, 512] -> a2a_in_h[slot, :, :]
            nc.sync.dma_start(
                a2a_in_h[h][:].rearrange("s p c -> p s c"),
                atall[:, h, :, :])
            nc.gpsimd.collective_compute(
                "AllToAll",
                mybir.AluOpType.bypass,
                ins=[a2a_in_h[h].opt()],
                outs=[a2a_out_h[h].opt()],
                replica_groups=[list(range(NCORES))],
            )

        # schedule: proj(b0); attn(h0,b0) exps overlap proj(b1) on ACT/PE;
        # A2A#1 (head 0) overlaps all of head 1's attention; A2A#2 tails.
        project_batch(0)
        attn_batch(0, 0)
        project_batch(1)
        attn_batch(0, 1)
        head_a2a(0)
        attn_batch(1, 0)
        attn_batch(1, 1)
        head_a2a(1)

        # ---- c_proj over the received [NX, TSL] block -----------------------
        wproj_bf = pool1.tile([128, KT, NX], BF16)
        for half in range(2):
            sl = slice(half * (KT // 2), (half + 1) * (KT // 2))
            nc.sync.dma_start(
                wproj_bf[:, sl, :],
                wproj[:, :].rearrange("(kt p) c -> p kt c", p=128)[:, sl, :])
        # feature rows of art: partition 0:64 <- head parity 0, 64:128 <- parity 1
        art_bf = pool1.tile([128, KT, TSL], BF16)
        for h in range(HPC):
            nc.sync.dma_start(
                art_bf[h * D:(h + 1) * D, :, :],
                a2a_out_h[h][:].rearrange("s p c -> p s c"))
        otall = pool1.tile([128, KT, TSL], F32)
        for ntile in range(KT):
            ps = psum.tile([128, 2, 512], F32, tag="mm")
            for ft in range(KT):
                nc.tensor.matmul(
                    ps[:, 0, :],
                    lhsT=wproj_bf[:, ft, ntile * 128:(ntile + 1) * 128],
                    rhs=art_bf[:, ft, :],
                    start=(ft == 0), stop=(ft == KT - 1),
                )
            if zero_proj_bias:
                nc.vector.tensor_copy(otall[:, ntile, :], ps[:, 0, :])
            else:
                nc.scalar.activation(
                    otall[:, ntile, :], ps[:, 0, :],
                    mybir.ActivationFunctionType.Identity, bias=bproj_t[:, ntile, :])
            nc.sync.dma_start(
                out_ext[ntile * 128:(ntile + 1) * 128, :], otall[:, ntile, :])

    nc.finalize()
    return nc


_CACHE = {}


def _get_nc(zero_attn_bias, zero_proj_bias):
    key = (zero_attn_bias, zero_proj_bias)
    if key not in _CACHE:
        _CACHE[key] = build(*key)
    return _CACHE[key]


def kernel(hidden_states, c_attn_w, c_attn_b, c_proj_w, c_proj_b, **extra):
    hidden_states = np.asarray(hidden_states, np.float32)
    c_attn_w = np.asarray(c_attn_w, np.float32)
    c_attn_b = np.asarray(c_attn_b, np.float32)
    c_proj_w = np.asarray(c_proj_w, np.float32)
    c_proj_b = np.asarray(c_proj_b, np.float32)

    zero_attn_bias = not np.any(c_attn_b)
    zero_proj_bias = not np.any(c_proj_b)
    nc = _get_nc(zero_attn_bias, zero_proj_bias)

    bf = ml_dtypes.bfloat16
    # [NX, B*S] pre-transposed hidden states in the kernel's compute dtype
    hsT = np.ascontiguousarray(hidden_states.reshape(B * S, NX).T).astype(bf)
    wproj_bf = np.ascontiguousarray(c_proj_w).astype(bf)
    bproj = np.ascontiguousarray(c_proj_b.reshape(NX, 1))

    in_maps = []
    for i in range(NCORES):
        cols = np.r_[i * GF:(i + 1) * GF,
                     NX + i * GF:NX + (i + 1) * GF,
                     2 * NX + i * GF:2 * NX + (i + 1) * GF]
        in_maps.append({
            "hst": hsT,
            "wqkv": np.ascontiguousarray(c_attn_w[:, cols]).astype(bf),
            "bqkv": np.ascontiguousarray(c_attn_b[cols].reshape(3 * GF, 1)),
            "wproj": wproj_bf,
            "bproj": bproj,
        })

    res = run_bass_kernel_spmd(nc, in_maps, core_ids=list(range(NCORES)))
    out = np.empty((B * S, NX), np.float32)
    for i in range(NCORES):
        out[i * TSL:(i + 1) * TSL, :] = res.results[i]["out"].T
    return out.reshape(B, S, NX)


if __name__ == "__main__":
    rng = np.random.default_rng(0)
    hs = rng.standard_normal((B, S, NX), dtype=np.float32)
    wa = (rng.standard_normal((NX, 3 * NX), dtype=np.float32) * 0.02)
    wp = (rng.standard_normal((NX, NX), dtype=np.float32) * 0.02)
    o = kernel(hidden_states=hs, c_attn_w=wa, c_attn_b=np.zeros(3 * NX, np.float32),
               c_proj_w=wp, c_proj_b=np.zeros(NX, np.float32))
    print(o.shape, o.dtype)



# revision 12
# speedup vs baseline: 1.3340x; 1.3340x over previous
"""Distributed GPT-2 attention block for 8 TRN2 NeuronCores.

Sharding: core i handles heads {2i, 2i+1} for BOTH batches (tensor-parallel
column split of c_attn). After attention, one 8-core AllToAll converts
head-sharding to token-sharding (512-token slice of the flattened [4096]
token axis per core), then each core runs c_proj (full 1024-feature
contraction) for its slice. Host unshard is pure concatenation.

Layout notes: hidden_states is passed pre-transposed [NX, B*S] (bf16, the
kernel's compute dtype) so q,k stay in [d, t] layout; scores are computed
as S^T = kT.T @ qT; the softmax denominator comes for free from an appended
ones-column in v during the PV matmul; causal masking = block skipping +
affine_select zeroing on diagonal tiles; exp and the 1/sqrt(d) scale are
fused into one ACT pass. All matmuls accumulate in f32 PSUM.
"""

import numpy as np
import ml_dtypes
from contextlib import ExitStack

import concourse.bass as bass
import concourse.bacc as bacc
import concourse.mybir as mybir
import concourse.tile as tile
from concourse.bass_utils import run_bass_kernel_spmd

B, S, NX = 2, 2048, 1024
H, D = 16, 64
HPC = 2              # heads per core
GF = HPC * D         # 128 features per head group
NCORES = 8
SF = B * S           # 4096 flattened tokens
TSL = SF // NCORES   # 512-token output slice per core

F32 = mybir.dt.float32
BF16 = mybir.dt.bfloat16


def build(zero_attn_bias: bool, zero_proj_bias: bool) -> bass.Bass:
    nc = bacc.Bacc(None)

    hst = nc.declare_dram_parameter("hst", [NX, SF], BF16, isOutput=False)
    wqkv = nc.declare_dram_parameter("wqkv", [NX, 3 * GF], BF16, isOutput=False)
    bqkv = nc.declare_dram_parameter("bqkv", [3 * GF, 1], F32, isOutput=False)
    wproj = nc.declare_dram_parameter("wproj", [NX, NX], BF16, isOutput=False)
    bproj = nc.declare_dram_parameter("bproj", [NX, 1], F32, isOutput=False)
    out_ext = nc.declare_dram_parameter("out", [NX, TSL], F32, isOutput=True)

    KT = NX // 128   # 8 k tiles

    with tile.TileContext(nc) as tc, ExitStack() as ctx:
        pool1 = ctx.enter_context(tc.tile_pool(name="persist", bufs=1))
        small = ctx.enter_context(tc.tile_pool(name="small", bufs=2))
        ppool = ctx.enter_context(tc.tile_pool(name="ppool", bufs=2))
        psum = ctx.enter_context(tc.tile_pool(name="psum", bufs=2, space="PSUM"))
        psum_av = ctx.enter_context(tc.tile_pool(name="psum_av", bufs=2, space="PSUM"))
        psum_rb = ctx.enter_context(tc.tile_pool(name="psum_rb", bufs=2, space="PSUM"))
        dram = ctx.enter_context(tc.tile_pool(name="dram", bufs=1, space="DRAM"))

        # ---- load weights and hidden states (bf16, direct, no staging) ------
        # 3D-AP DMAs: [part, ktile, col] <- DRAM[ktile*128 + part, col]
        wqkv_bf = pool1.tile([128, KT, 3 * GF], BF16)
        nc.sync.dma_start(
            wqkv_bf[:], wqkv[:, :].rearrange("(kt p) c -> p kt c", p=128))
        hst_bf = pool1.tile([128, KT, SF], BF16)
        for q in range(4):
            tsl = slice(q * (SF // 4), (q + 1) * (SF // 4))
            for kt in range(KT):
                nc.sync.dma_start(
                    hst_bf[:, kt, tsl], hst[kt * 128:(kt + 1) * 128, tsl])

        # biases as per-partition tiles (q/k: feature-per-partition in qkT layout)
        bqk_t = pool1.tile([128, 2, 1], F32)   # ft 0 = q(2 heads), ft 1 = k
        if not zero_attn_bias:
            for ft in range(2):
                nc.sync.dma_start(bqk_t[:, ft, :], bqkv[ft * 128:(ft + 1) * 128, :])
        bv_t = pool1.tile([64, HPC, 1], F32)
        if not zero_attn_bias:
            for h in range(HPC):
                nc.sync.dma_start(
                    bv_t[:, h, :], bqkv[2 * GF + h * D:2 * GF + (h + 1) * D, :])
        bproj_t = pool1.tile([128, KT, 1], F32)
        if not zero_proj_bias:
            nc.sync.dma_start(
                bproj_t[:], bproj[:, :].rearrange("(kt p) c -> p kt c", p=128))

        # ---- QKV projection --------------------------------------------------
        # q,k transposed: qk_sb[:, ft, t]; ft 0 = q (2 heads), ft 1 = k
        qk_sb = pool1.tile([128, 2, SF], BF16)
        v_sb = pool1.tile([128, SF // 128, HPC, D + 1], BF16)

        def project_batch(b):
            for ft in range(2):
                for tch in range(4 * b, 4 * b + 4):
                    ps = psum.tile([128, 2, 512], F32, tag="mm")
                    for kt in range(KT):
                        nc.tensor.matmul(
                            ps[:, 0, :],
                            lhsT=wqkv_bf[:, kt, ft * 128:(ft + 1) * 128],
                            rhs=hst_bf[:, kt, tch * 512:(tch + 1) * 512],
                            start=(kt == 0), stop=(kt == KT - 1),
                        )
                    if zero_attn_bias:
                        nc.vector.tensor_copy(
                            qk_sb[:, ft, tch * 512:(tch + 1) * 512], ps[:, 0, :])
                    else:
                        nc.scalar.activation(
                            qk_sb[:, ft, tch * 512:(tch + 1) * 512], ps[:, 0, :],
                            mybir.ActivationFunctionType.Identity,
                            bias=bqk_t[:, ft, :])
            # v natural [token, feat] + ones column: v_sb [128, tt, h, 65]
            for tt in range(16 * b, 16 * b + 16):
                ps = psum.tile([128, 2, 512], F32, tag="mm")
                for kt in range(KT):
                    nc.tensor.matmul(
                        ps[:, 0, 0:GF],
                        lhsT=hst_bf[:, kt, tt * 128:(tt + 1) * 128],
                        rhs=wqkv_bf[:, kt, 2 * GF:3 * GF],
                        start=(kt == 0), stop=(kt == KT - 1),
                    )
                nc.vector.tensor_copy(
                    v_sb[:, tt, :, 0:D],
                    ps[:, 0, 0:GF].rearrange("p (h d) -> p h d", h=HPC))
            nc.gpsimd.memset(v_sb[:, 16 * b:16 * b + 16, :, D:D + 1], 1.0)

        # ---- attention -------------------------------------------------------
        # one A2A per head: the h=0 collective overlaps h=1 attention compute
        a2a_in_h = [dram.tile([NCORES, D, TSL], BF16, name=f"a2ain{h}")
                    for h in range(HPC)]
        a2a_out_h = [dram.tile([NCORES, D, TSL], BF16, name=f"a2aout{h}")
                     for h in range(HPC)]

        ones1 = pool1.tile([1, D + 1], BF16)
        nc.gpsimd.memset(ones1[:], 1.0)
        # shifted triangular mask strip: strip[p, x] = 1.0 iff x - 384 >= p
        strip = pool1.tile([128, 896], BF16)
        nc.gpsimd.memset(strip[:], 1.0)
        nc.gpsimd.affine_select(
            out=strip[:], in_=strip[:],
            compare_op=mybir.AluOpType.is_ge, fill=0.0,
            base=-384, pattern=[[1, 896]], channel_multiplier=-1)

        # staging for all heads'/blocks' normalized attention outputs:
        # [64 part, h, slot(=4b+tb), 512] -> one DMA per h to a2a_in
        atall = pool1.tile([D, HPC, NCORES, TSL], BF16)

        pending = []

        def attn_batch(h, b):
            qrow = 64 * h
            if True:
                tok0 = b * S               # batch token offset (flattened)
                tt0 = tok0 // 128          # v tile offset
                for tb in (3, 2, 1, 0):    # big blocks first: short tail chain
                    ntj = 4 * (tb + 1)     # causal: tj tiles 0..ntj-1
                    av = psum_av.tile([D + 1, 512], F32, tag="av")
                    for tjq in range(ntj // 4):   # quads of tj tiles
                        if tjq < ntj // 4 - 1:
                            # full (strictly-lower) quad
                            st_ps = psum.tile([128, 2, 512], F32, tag="mm")
                            pt = ppool.tile([128, 4, 512], BF16, tag="pt")
                            for u in range(4):
                                t = 4 * tjq + u
                                nc.tensor.matmul(
                                    st_ps[:, u % 2, :],
                                    lhsT=qk_sb[qrow:qrow + 64, 1,
                                               tok0 + t * 128:tok0 + (t + 1) * 128],
                                    rhs=qk_sb[qrow:qrow + 64, 0,
                                              tok0 + tb * 512:tok0 + (tb + 1) * 512],
                                    start=True, stop=True,
                                )
                                # exp((q.k)/sqrt(d)); pairs share one psum tile
                                if u % 2 == 1:
                                    nc.scalar.activation(
                                        pt[:, u - 1:u + 1, :], st_ps[:],
                                        mybir.ActivationFunctionType.Exp, scale=0.125)
                                    if u == 1:
                                        st_ps = psum.tile([128, 2, 512], F32, tag="mm")
                            for u in range(4):
                                t = 4 * tjq + u
                                nc.tensor.matmul(
                                    av[:],
                                    lhsT=v_sb[:, tt0 + t, h, :],
                                    rhs=pt[:, u, :],
                                    start=(t == 0), stop=(t == ntj - 1),
                                )
                        else:
                            # diagonal quad: tile u only needs queries
                            # ti >= 128u -> widths 512/384/256/128, packed in
                            # pairs; masked via strip multiply (keep c' >= p)
                            for pair in range(2):
                                st_d = psum.tile([128, 1024], F32, tag="mm")
                                pt_d = ppool.tile([128, 1024], BF16, tag="pt")
                                w0 = 512 - 128 * (2 * pair)
                                w1 = 512 - 128 * (2 * pair + 1)
                                for u2 in range(2):
                                    u = 2 * pair + u2
                                    t = 4 * tjq + u
                                    w = 512 - 128 * u
                                    off = 0 if u2 == 0 else w0
                                    qlo = tok0 + tb * 512 + 128 * u
                                    nc.tensor.matmul(
                                        st_d[:, off:off + w],
                                        lhsT=qk_sb[qrow:qrow + 64, 1,
                                                   tok0 + t * 128:tok0 + (t + 1) * 128],
                                        rhs=qk_sb[qrow:qrow + 64, 0, qlo:qlo + w],
                                        start=True, stop=True,
                                    )
                                nc.scalar.activation(
                                    pt_d[:, 0:w0 + w1], st_d[:, 0:w0 + w1],
                                    mybir.ActivationFunctionType.Exp, scale=0.125)
                                for u2 in range(2):
                                    u = 2 * pair + u2
                                    t = 4 * tjq + u
                                    w = 512 - 128 * u
                                    off = 0 if u2 == 0 else w0
                                    nc.vector.tensor_mul(
                                        pt_d[:, off:off + w], pt_d[:, off:off + w],
                                        strip[:, 384:384 + w])
                                    nc.tensor.matmul(
                                        av[:, 128 * u:512],
                                        lhsT=v_sb[:, tt0 + t, h, :],
                                        rhs=pt_d[:, off:off + w],
                                        start=(t == 0), stop=(t == ntj - 1),
                                    )
                        if pending:
                            pending.pop(0)()
                    # normalize by row D (the P row-sums); recip now (DVE),
                    # but defer the PE broadcast+mul until after the next
                    # block's S quads so PE never waits on DVE here
                    # custom-DVE reciprocal requires base partition 0: stage
                    # the denominator row (PSUM partition 64) down to p0 first
                    den = small.tile([1, 512], F32, tag="den")
                    nc.vector.tensor_copy(den[:], av[D:D + 1, :])
                    recipf = small.tile([1, 512], F32, tag="recipf")
                    nc.vector.reciprocal_approx_fast(recipf[:], den[:])
                    recip = small.tile([1, 512], BF16, tag="recip")
                    with nc.allow_low_precision("softmax recip bf16 is fine"):
                        nc.vector.tensor_copy(recip[:], recipf[:])

                    def make_epilogue(av=av, recip=recip, h=h, slot=4 * b + tb):
                        def epi():
                            rb = psum_rb.tile([D, 512], F32, tag="rb")
                            nc.tensor.matmul(rb[:], lhsT=ones1[:, 0:D],
                                             rhs=recip[:], start=True, stop=True)
                            rb_sb = ppool.tile([D, 512], F32, tag="rbs")
                            nc.vector.tensor_copy(rb_sb[:], rb[:])
                            dst = atall[:, h, slot, :]
                            if zero_attn_bias:
                                nc.vector.tensor_mul(dst, av[0:D, :], rb_sb[:])
                            else:
                                at = ppool.tile([D, 512], BF16, tag="at")
                                nc.vector.tensor_mul(at[:], av[0:D, :], rb_sb[:])
                                nc.scalar.activation(
                                    dst, at[:],
                                    mybir.ActivationFunctionType.Identity,
                                    bias=bv_t[:, h, :])
                        return epi
                    pending.append(make_epilogue())

        def head_a2a(h):
            while pending:
                pending.pop(0)()
            # per-head store + AllToAll: [64, slot, 512] -> a2a_in_h[slot, :, :]
            nc.sync.dma_start(
                a2a_in_h[h][:].rearrange("s p c -> p s c"),
                atall[:, h, :, :])
            nc.gpsimd.collective_compute(
                "AllToAll",
                mybir.AluOpType.bypass,
                ins=[a2a_in_h[h].opt()],
                outs=[a2a_out_h[h].opt()],
                replica_groups=[list(range(NCORES))],
            )

        # schedule: proj(b0); attn(h0,b0) exps overlap proj(b1) on ACT/PE;
        # A2A#1 (head 0) overlaps all of head 1's attention; A2A#2 tails.
        project_batch(0)
        attn_batch(0, 0)
        project_batch(1)
        attn_batch(0, 1)
        head_a2a(0)
        attn_batch(1, 0)
        attn_batch(1, 1)
        head_a2a(1)

        # ---- c_proj over the received [NX, TSL] block -----------------------
        wproj_bf = pool1.tile([128, KT, NX], BF16)
        for half in range(2):
            sl = slice(half * (KT // 2), (half + 1) * (KT // 2))
            nc.sync.dma_start(
                wproj_bf[:, sl, :],
                wproj[:, :].rearrange("(kt p) c -> p kt c", p=128)[:, sl, :])
        # feature rows of art: partition 0:64 <- head parity 0, 64:128 <- parity 1
        art_bf = pool1.tile([128, KT, TSL], BF16)
        for h in range(HPC):
            nc.sync.dma_start(
                art_bf[h * D:(h + 1) * D, :, :],
                a2a_out_h[h][:].rearrange("s p c -> p s c"))
        otall = pool1.tile([128, KT, TSL], F32)
        for ntile in range(KT):
            ps = psum.tile([128, 2, 512], F32, tag="mm")
            for ft in range(KT):
                nc.tensor.matmul(
                    ps[:, 0, :],
                    lhsT=wproj_bf[:, ft, ntile * 128:(ntile + 1) * 128],
                    rhs=art_bf[:, ft, :],
                    start=(ft == 0), stop=(ft == KT - 1),
                )
            if zero_proj_bias:
                nc.vector.tensor_copy(otall[:, ntile, :], ps[:, 0, :])
            else:
                nc.scalar.activation(
                    otall[:, ntile, :], ps[:, 0, :],
                    mybir.ActivationFunctionType.Identity, bias=bproj_t[:, ntile, :])
            nc.sync.dma_start(
                out_ext[ntile * 128:(ntile + 1) * 128, :], otall[:, ntile, :])

    nc.finalize()
    return nc


_CACHE = {}


def _get_nc(zero_attn_bias, zero_proj_bias):
    key = (zero_attn_bias, zero_proj_bias)
    if key not in _CACHE:
        _CACHE[key] = build(*key)
    return _CACHE[key]


def kernel(hidden_states, c_attn_w, c_attn_b, c_proj_w, c_proj_b, **extra):
    hidden_states = np.asarray(hidden_states, np.float32)
    c_attn_w = np.asarray(c_attn_w, np.float32)
    c_attn_b = np.asarray(c_attn_b, np.float32)
    c_proj_w = np.asarray(c_proj_w, np.float32)
    c_proj_b = np.asarray(c_proj_b, np.float32)

    zero_attn_bias = not np.any(c_attn_b)
    zero_proj_bias = not np.any(c_proj_b)
    nc = _get_nc(zero_attn_bias, zero_proj_bias)

    bf = ml_dtypes.bfloat16
    # [NX, B*S] pre-transposed hidden states in the kernel's compute dtype
    hsT = np.ascontiguousarray(hidden_states.reshape(B * S, NX).T).astype(bf)
    wproj_bf = np.ascontiguousarray(c_proj_w).astype(bf)
    bproj = np.ascontiguousarray(c_proj_b.reshape(NX, 1))

    in_maps = []
    for i in range(NCORES):
        cols = np.r_[i * GF:(i + 1) * GF,
                     NX + i * GF:NX + (i + 1) * GF,
                     2 * NX + i * GF:2 * NX + (i + 1) * GF]
        in_maps.append({
            "hst": hsT,
            "wqkv": np.ascontiguousarray(c_attn_w[:, cols]).astype(bf),
            "bqkv": np.ascontiguousarray(c_attn_b[cols].reshape(3 * GF, 1)),
            "wproj": wproj_bf,
            "bproj": bproj,
        })

    res = run_bass_kernel_spmd(nc, in_maps, core_ids=list(range(NCORES)))
    out = np.empty((B * S, NX), np.float32)
    for i in range(NCORES):
        out[i * TSL:(i + 1) * TSL, :] = res.results[i]["out"].T
    return out.reshape(B, S, NX)


if __name__ == "__main__":
    rng = np.random.default_rng(0)
    hs = rng.standard_normal((B, S, NX), dtype=np.float32)
    wa = (rng.standard_normal((NX, 3 * NX), dtype=np.float32) * 0.02)
    wp = (rng.standard_normal((NX, NX), dtype=np.float32) * 0.02)
    o = kernel(hidden_states=hs, c_attn_w=wa, c_attn_b=np.zeros(3 * NX, np.float32),
               c_proj_w=wp, c_proj_b=np.zeros(NX, np.float32))
    print(o.shape, o.dtype)



# revision 19
# speedup vs baseline: 1.3354x; 1.0010x over previous
"""Distributed GPT-2 attention block for 8 TRN2 NeuronCores.

Sharding: core i handles heads {2i, 2i+1} for BOTH batches (tensor-parallel
column split of c_attn). After attention, one 8-core AllToAll per head
converts head-sharding to token-sharding (512-token slice of the flattened
[4096] token axis per core), then each core runs c_proj (full 1024-feature
contraction) for its slice. Host unshard is pure concatenation.

Schedule (v2): engines are kept co-busy by interleaving independent
instruction streams (TRN2's PE p-state throttles 2-3.7x after idle gaps, so
PE continuity is the top priority):
  phase 1: QKV projection of batch 0 (PE solid; qk casts on ACT, v on DVE)
  phase 2: attention units (h0,b0)+(h1,b0) interleaved quad-by-quad, with
           batch-1 QKV projection tasks as PE filler between quads
  phase 3: units (h0,b1)+(h1,b1) interleaved 2:1 so head 0 finishes early;
           its AllToAll then overlaps the rest of head 1's attention
  tail:    AllToAll(h1) overlaps the parity-0 half of c_proj (contraction
           rows 0:64 = head-parity-0 features arrive in AllToAll#1); the
           parity-1 half + combine + store runs after AllToAll#2 lands.

Numerics: scores are computed as S^T = kT.T @ qT in bf16 with f32 PSUM; the
softmax denominator comes free from an appended ones-column in v during the
PV matmul; exp and the 1/sqrt(d) scale are fused into one ACT pass; the
denominator reciprocal uses the fast custom-DVE approximation (applied to
the whole [65,512] AV tile because the custom op requires base partition 0)
and is broadcast across partitions with a ones-column matmul.
"""

import numpy as np
import ml_dtypes
from contextlib import ExitStack

import concourse.bass as bass
import concourse.bacc as bacc
import concourse.mybir as mybir
import concourse.tile as tile
from concourse.bass_utils import run_bass_kernel_spmd

B, S, NX = 2, 2048, 1024
H, D = 16, 64
HPC = 2              # heads per core
GF = HPC * D         # 128 features per head group
NCORES = 8
SF = B * S           # 4096 flattened tokens
TSL = SF // NCORES   # 512-token output slice per core

F32 = mybir.dt.float32
BF16 = mybir.dt.bfloat16
IDENT = mybir.ActivationFunctionType.Identity
EXP = mybir.ActivationFunctionType.Exp


def build(zero_attn_bias: bool, zero_proj_bias: bool) -> bass.Bass:
    nc = bacc.Bacc(None)

    hst = nc.declare_dram_parameter("hst", [NX, SF], BF16, isOutput=False)
    wqkv = nc.declare_dram_parameter("wqkv", [NX, 3 * GF], BF16, isOutput=False)
    bqkv = nc.declare_dram_parameter("bqkv", [3 * GF, 1], F32, isOutput=False)
    wproj = nc.declare_dram_parameter("wproj", [NX, NX], BF16, isOutput=False)
    bproj = nc.declare_dram_parameter("bproj", [NX, 1], F32, isOutput=False)
    out_ext = nc.declare_dram_parameter("out", [NX, TSL], F32, isOutput=True)

    KT = NX // 128   # 8 k tiles

    with tile.TileContext(nc) as tc, ExitStack() as ctx:
        pool1 = ctx.enter_context(tc.tile_pool(name="persist", bufs=1))
        small = ctx.enter_context(tc.tile_pool(name="small", bufs=2))
        ppool = ctx.enter_context(tc.tile_pool(name="ppool", bufs=2))
        psum = ctx.enter_context(tc.tile_pool(name="psum", bufs=2, space="PSUM"))
        psum_av = ctx.enter_context(tc.tile_pool(name="psum_av", bufs=2, space="PSUM"))
        psum_rb = ctx.enter_context(tc.tile_pool(name="psum_rb", bufs=1, space="PSUM"))
        psum_pm = ctx.enter_context(tc.tile_pool(name="psum_pm", bufs=1, space="PSUM"))
        dram = ctx.enter_context(tc.tile_pool(name="dram", bufs=1, space="DRAM"))

        # ---- load weights and hidden states (bf16, direct, no staging) ------
        wqkv_bf = pool1.tile([128, KT, 3 * GF], BF16)
        nc.sync.dma_start(
            wqkv_bf[:], wqkv[:, :].rearrange("(kt p) c -> p kt c", p=128))
        hst_bf = pool1.tile([128, KT, SF], BF16)
        for q in range(4):
            tsl = slice(q * (SF // 4), (q + 1) * (SF // 4))
            eng = nc.sync if q < 2 else nc.gpsimd
            for kt in range(KT):
                eng.dma_start(
                    hst_bf[:, kt, tsl], hst[kt * 128:(kt + 1) * 128, tsl])

        # biases as per-partition tiles (q/k: feature-per-partition in qkT layout)
        bqk_t = pool1.tile([128, 2, 1], F32)   # ft 0 = q(2 heads), ft 1 = k
        if not zero_attn_bias:
            for ft in range(2):
                nc.sync.dma_start(bqk_t[:, ft, :], bqkv[ft * 128:(ft + 1) * 128, :])
        bv_t = pool1.tile([64, HPC, 1], F32)
        if not zero_attn_bias:
            for h in range(HPC):
                nc.sync.dma_start(
                    bv_t[:, h, :], bqkv[2 * GF + h * D:2 * GF + (h + 1) * D, :])
        bproj_t = pool1.tile([128, KT, 1], F32)
        if not zero_proj_bias:
            nc.sync.dma_start(
                bproj_t[:], bproj[:, :].rearrange("(kt p) c -> p kt c", p=128))

        # ---- persistent SBUF state ------------------------------------------
        # q,k transposed: qk_sb[:, ft, t]; ft 0 = q (2 heads), ft 1 = k
        qk_sb = pool1.tile([128, 2, SF], BF16)
        v_sb = pool1.tile([128, SF // 128, HPC, D + 1], BF16)
        nc.gpsimd.memset(v_sb[:, :, :, D:D + 1], 1.0)

        ones1 = pool1.tile([1, D], BF16)
        nc.gpsimd.memset(ones1[:], 1.0)
        # shifted triangular mask strip: strip[p, x] = 1.0 iff x - 384 >= p
        strip = pool1.tile([128, 896], BF16)
        nc.gpsimd.memset(strip[:], 1.0)
        nc.gpsimd.affine_select(
            out=strip[:], in_=strip[:],
            compare_op=mybir.AluOpType.is_ge, fill=0.0,
            base=-384, pattern=[[1, 896]], channel_multiplier=-1)

        # staging for all heads'/blocks' normalized attention outputs:
        # [64 part, h, slot(=4b+tb), 512] -> one DMA per h to a2a_in
        atall = pool1.tile([D, HPC, NCORES, TSL], BF16)

        a2a_in_h = [dram.tile([NCORES, D, TSL], BF16, name=f"a2ain{h}")
                    for h in range(HPC)]
        a2a_out_h = [dram.tile([NCORES, D, TSL], BF16, name=f"a2aout{h}")
                     for h in range(HPC)]

        # ---- QKV projection task generator ----------------------------------
        # yields one closure per PSUM-chain; alternates psum tags "mm"/"pm"
        # so consecutive chains overlap.
        def gen_proj(b):
            tagtog = [0]

            def take_ps():
                tagtog[0] ^= 1
                if tagtog[0]:
                    t = psum.tile([128, 2, 512], F32, tag="mm", name="projps")
                    return t[:, 0, :]
                t = psum_pm.tile([128, 512], F32, tag="pm", name="projps")
                return t[:, :]

            def make_qk(ft, tch):
                def qk_task():
                    ps = take_ps()
                    for kt in range(KT):
                        nc.tensor.matmul(
                            ps,
                            lhsT=wqkv_bf[:, kt, ft * 128:(ft + 1) * 128],
                            rhs=hst_bf[:, kt, tch * 512:(tch + 1) * 512],
                            start=(kt == 0), stop=(kt == KT - 1),
                        )
                    dst = qk_sb[:, ft, tch * 512:(tch + 1) * 512]
                    if b == 0:
                        # ACT is idle during phase 1
                        if zero_attn_bias:
                            nc.scalar.activation(dst, ps, IDENT)
                        else:
                            nc.scalar.activation(dst, ps, IDENT,
                                                 bias=bqk_t[:, ft, :])
                    else:
                        if zero_attn_bias:
                            nc.vector.tensor_copy(dst, ps)
                        else:
                            nc.scalar.activation(dst, ps, IDENT,
                                                 bias=bqk_t[:, ft, :])
                return qk_task

            def make_v(tt):
                def v_task():
                    ps = take_ps()
                    for kt in range(KT):
                        nc.tensor.matmul(
                            ps[:, 0:GF],
                            lhsT=hst_bf[:, kt, tt * 128:(tt + 1) * 128],
                            rhs=wqkv_bf[:, kt, 2 * GF:3 * GF],
                            start=(kt == 0), stop=(kt == KT - 1),
                        )
                    nc.vector.tensor_copy(
                        v_sb[:, tt, :, 0:D],
                        ps[:, 0:GF].rearrange("p (h d) -> p h d", h=HPC))
                return v_task

            # interleave qk (long) and v (short) tasks so filler slots give
            # roughly even PE padding
            qk_tasks = [make_qk(ft, tch)
                        for ft in range(2) for tch in range(4 * b, 4 * b + 4)]
            v_tasks = [make_v(tt) for tt in range(16 * b, 16 * b + 16)]
            for i in range(8):
                yield qk_tasks[i]
                yield v_tasks[2 * i]
                yield v_tasks[2 * i + 1]

        # ---- attention unit task generator ----------------------------------
        # unit = (head h, batch b); blocks tb 3..0, each with tb full quads
        # and one diagonal quad; yields S (scores+exp) and V (PV) tasks per
        # quad, then den (reciprocal) and epi (normalize into atall) tasks.
        def gen_unit(h, b):
            qrow = 64 * h
            tok0 = b * S
            tt0 = tok0 // 128
            for tb in (3, 2, 1, 0):
                ntj = 4 * (tb + 1)
                box = {}

                def open_av(box=box):
                    if "av" not in box:
                        box["av"] = psum_av.tile([D + 1, 512], F32, tag="av", name="av")
                    return box["av"]

                for tjq in range(tb + 1):
                    if tjq < tb:
                        # full (strictly-lower) quad: 4 scores mm + 2 exps
                        def s_full(tjq=tjq, tb=tb, box=box):
                            pt = ppool.tile([128, 4, 512], BF16, tag="pt", name="pt")
                            box["pt"] = pt
                            for pair in range(2):
                                st = psum.tile([128, 2, 512], F32, tag="mm", name="st")
                                for u2 in range(2):
                                    u = 2 * pair + u2
                                    t = 4 * tjq + u
                                    nc.tensor.matmul(
                                        st[:, u2, :],
                                        lhsT=qk_sb[qrow:qrow + 64, 1,
                                                   tok0 + t * 128:tok0 + (t + 1) * 128],
                                        rhs=qk_sb[qrow:qrow + 64, 0,
                                                  tok0 + tb * 512:tok0 + (tb + 1) * 512],
                                        start=True, stop=True,
                                    )
                                nc.scalar.activation(
                                    pt[:, 2 * pair:2 * pair + 2, :], st[:],
                                    EXP, scale=0.125)

                        def v_full(tjq=tjq, tb=tb, box=box):
                            av = open_av()
                            pt = box["pt"]
                            for u in range(4):
                                t = 4 * tjq + u
                                nc.tensor.matmul(
                                    av[:],
                                    lhsT=v_sb[:, tt0 + t, h, :],
                                    rhs=pt[:, u, :],
                                    start=(t == 0), stop=(t == ntj - 1),
                                )
                        yield s_full
                        yield v_full
                    else:
                        # diagonal quad: tile u only needs queries >= 128u;
                        # widths 512/384/256/128 packed in pairs, masked by
                        # strip multiply (keep c' >= p)
                        def s_diag(tjq=tjq, tb=tb, box=box):
                            pt_d = ppool.tile([128, 2, 1024], BF16, tag="ptd", name="pt_d")
                            box["ptd"] = pt_d
                            for pair in range(2):
                                st_d = psum.tile([128, 1024], F32, tag="mm", name="st_d")
                                w0 = 512 - 128 * (2 * pair)
                                w1 = 512 - 128 * (2 * pair + 1)
                                for u2 in range(2):
                                    u = 2 * pair + u2
                                    t = 4 * tjq + u
                                    w = 512 - 128 * u
                                    off = 0 if u2 == 0 else w0
                                    qlo = tok0 + tb * 512 + 128 * u
                                    nc.tensor.matmul(
                                        st_d[:, off:off + w],
                                        lhsT=qk_sb[qrow:qrow + 64, 1,
                                                   tok0 + t * 128:tok0 + (t + 1) * 128],
                                        rhs=qk_sb[qrow:qrow + 64, 0, qlo:qlo + w],
                                        start=True, stop=True,
                                    )
                                nc.scalar.activation(
                                    pt_d[:, pair, 0:w0 + w1],
                                    st_d[:, 0:w0 + w1], EXP, scale=0.125)
                                for u2 in range(2):
                                    u = 2 * pair + u2
                                    w = 512 - 128 * u
                                    off = 0 if u2 == 0 else w0
                                    nc.vector.tensor_mul(
                                        pt_d[:, pair, off:off + w],
                                        pt_d[:, pair, off:off + w],
                                        strip[:, 384:384 + w])

                        def v_diag(tjq=tjq, tb=tb, box=box):
                            av = open_av()
                            pt_d = box["ptd"]
                            for pair in range(2):
                                w0 = 512 - 128 * (2 * pair)
                                for u2 in range(2):
                                    u = 2 * pair + u2
                                    t = 4 * tjq + u
                                    w = 512 - 128 * u
                                    off = 0 if u2 == 0 else w0
                                    nc.tensor.matmul(
                                        av[:, 128 * u:512],
                                        lhsT=v_sb[:, tt0 + t, h, :],
                                        rhs=pt_d[:, pair, off:off + w],
                                        start=(t == 0), stop=(t == ntj - 1),
                                    )
                        yield s_diag
                        yield v_diag

                def den_task(box=box, tb=tb):
                    av = box["av"]
                    # custom-DVE reciprocal requires base partition 0: apply
                    # it to the whole [65,512] AV tile (cost is free-dim
                    # bound); only row D (the denominator row-sums) is used.
                    recipf = small.tile([D + 1, 512], F32, tag="recipf", name="recipf")
                    nc.vector.reciprocal_approx_fast(recipf[:], av[:])
                    recip = small.tile([1, 512], BF16, tag="recip", name="recip")
                    with nc.allow_low_precision("softmax recip bf16 is fine"):
                        nc.vector.tensor_copy(recip[:], recipf[D:D + 1, :])
                    box["recip"] = recip

                def epi_task(box=box, h=h, slot=4 * b + tb):
                    av, recip = box["av"], box["recip"]
                    rb = psum_rb.tile([D, 512], F32, tag="rb", name="rb")
                    nc.tensor.matmul(rb[:], lhsT=ones1[:], rhs=recip[:],
                                     start=True, stop=True)
                    rb_sb = ppool.tile([D, 512], F32, tag="rbs", name="rb_sb")
                    nc.vector.tensor_copy(rb_sb[:], rb[:])
                    dst = atall[:, h, slot, :]
                    if zero_attn_bias:
                        nc.vector.tensor_mul(dst, av[0:D, :], rb_sb[:])
                    else:
                        at = ppool.tile([D, 512], BF16, tag="at", name="at")
                        nc.vector.tensor_mul(at[:], av[0:D, :], rb_sb[:])
                        nc.scalar.activation(dst, at[:], IDENT,
                                             bias=bv_t[:, h, :])
                yield den_task
                yield epi_task

        def head_a2a(h):
            # per-head store + AllToAll: [64, slot, 512] -> a2a_in_h[slot, :, :]
            nc.sync.dma_start(
                a2a_in_h[h][:].rearrange("s p c -> p s c"),
                atall[:, h, :, :])
            nc.gpsimd.collective_compute(
                "AllToAll",
                mybir.AluOpType.bypass,
                ins=[a2a_in_h[h].opt()],
                outs=[a2a_out_h[h].opt()],
                replica_groups=[list(range(NCORES))],
            )

        # art: feature rows, partition 0:64 <- head parity 0, 64:128 <- parity 1
        art_bf = pool1.tile([128, KT, TSL], BF16)

        def art_dma(h, eng):
            eng.dma_start(
                art_bf[h * D:(h + 1) * D, :, :],
                a2a_out_h[h][:].rearrange("s p c -> p s c"))

        # ---- emission -------------------------------------------------------
        # `order` is a cyclic schedule of stream indices, e.g. [0, 1, 0]
        # pulls stream 0 twice per round with stream 1 between the pulls.
        def run_zip(streams, order, on_exhaust=None):
            live = [iter(s) for s in streams]
            done = [False] * len(streams)
            while not all(done):
                for i in order:
                    if done[i]:
                        continue
                    try:
                        next(live[i])()
                    except StopIteration:
                        done[i] = True
                        if on_exhaust is not None:
                            on_exhaust(i)

        # phase 1: projection of batch 0 (PE solid, ACT does the qk casts)
        for task in gen_proj(0):
            task()

        # phase 2: batch-0 attention for both heads + batch-1 projection filler
        run_zip([gen_unit(0, 0), gen_unit(1, 0), gen_proj(1)], [0, 1, 2])

        # wproj load for the c_proj tail (issued early, off the critical path)
        wproj_bf = pool1.tile([128, KT, NX], BF16)
        for half in range(2):
            sl = slice(half * (KT // 2), (half + 1) * (KT // 2))
            nc.gpsimd.dma_start(
                wproj_bf[:, sl, :],
                wproj[:, :].rearrange("(kt p) c -> p kt c", p=128)[:, sl, :])

        # phase 3: batch-1 attention, head 0 prioritized 2:1 so its AllToAll
        # fires while head 1 is still computing.  art0's DMA is issued from
        # the Pool queue (right behind the collective) so it doesn't
        # head-of-line-block the sync queue that atall-h1 needs.
        def fire_a2a0(i):
            if i == 0:
                head_a2a(0)
                art_dma(0, nc.gpsimd)
        run_zip([gen_unit(0, 1), gen_unit(1, 1)], [0, 1, 0], on_exhaust=fire_a2a0)

        head_a2a(1)
        art_dma(1, nc.sync)

        # ---- c_proj: parity-0 contraction half overlaps AllToAll#2 ----------
        otall0 = pool1.tile([128, KT, TSL], F32)
        for ntile in range(KT):
            if ntile % 2 == 0:
                ps0 = psum_pm.tile([128, 512], F32, tag="pm", name="ps0")
            else:
                ps0full = psum.tile([128, 2, 512], F32, tag="mm", name="ps0")
                ps0 = ps0full[:, 0, :]
            for ft in range(KT):
                nc.tensor.matmul(
                    ps0[:],
                    lhsT=wproj_bf[0:64, ft, ntile * 128:(ntile + 1) * 128],
                    rhs=art_bf[0:64, ft, :],
                    start=(ft == 0), stop=(ft == KT - 1),
                )
            nc.scalar.activation(otall0[:, ntile, :], ps0[:], IDENT)

        otall = pool1.tile([128, KT, TSL], F32)
        for ntile in range(KT):
            ps1 = psum.tile([128, 2, 512], F32, tag="mm", name="ps1")
            for ft in range(KT):
                nc.tensor.matmul(
                    ps1[:, 0, :],
                    lhsT=wproj_bf[64:128, ft, ntile * 128:(ntile + 1) * 128],
                    rhs=art_bf[64:128, ft, :],
                    start=(ft == 0), stop=(ft == KT - 1),
                )
            nc.vector.tensor_add(otall[:, ntile, :], ps1[:, 0, :],
                                 otall0[:, ntile, :])
            if not zero_proj_bias:
                nc.scalar.activation(otall[:, ntile, :], otall[:, ntile, :],
                                     IDENT, bias=bproj_t[:, ntile, :])
            nc.sync.dma_start(
                out_ext[ntile * 128:(ntile + 1) * 128, :], otall[:, ntile, :])

    nc.finalize()
    return nc


_CACHE = {}


def _get_nc(zero_attn_bias, zero_proj_bias):
    key = (zero_attn_bias, zero_proj_bias)
    if key not in _CACHE:
        _CACHE[key] = build(*key)
    return _CACHE[key]


def kernel(hidden_states, c_attn_w, c_attn_b, c_proj_w, c_proj_b, **extra):
    hidden_states = np.asarray(hidden_states, np.float32)
    c_attn_w = np.asarray(c_attn_w, np.float32)
    c_attn_b = np.asarray(c_attn_b, np.float32)
    c_proj_w = np.asarray(c_proj_w, np.float32)
    c_proj_b = np.asarray(c_proj_b, np.float32)

    zero_attn_bias = not np.any(c_attn_b)
    zero_proj_bias = not np.any(c_proj_b)
    nc = _get_nc(zero_attn_bias, zero_proj_bias)

    bf = ml_dtypes.bfloat16
    # [NX, B*S] pre-transposed hidden states in the kernel's compute dtype
    hsT = np.ascontiguousarray(hidden_states.reshape(B * S, NX).T).astype(bf)
    wproj_bf = np.ascontiguousarray(c_proj_w).astype(bf)
    bproj = np.ascontiguousarray(c_proj_b.reshape(NX, 1))

    in_maps = []
    for i in range(NCORES):
        cols = np.r_[i * GF:(i + 1) * GF,
                     NX + i * GF:NX + (i + 1) * GF,
                     2 * NX + i * GF:2 * NX + (i + 1) * GF]
        in_maps.append({
            "hst": hsT,
            "wqkv": np.ascontiguousarray(c_attn_w[:, cols]).astype(bf),
            "bqkv": np.ascontiguousarray(c_attn_b[cols].reshape(3 * GF, 1)),
            "wproj": wproj_bf,
            "bproj": bproj,
        })

    res = run_bass_kernel_spmd(nc, in_maps, core_ids=list(range(NCORES)))
    out = np.empty((B * S, NX), np.float32)
    for i in range(NCORES):
        out[i * TSL:(i + 1) * TSL, :] = res.results[i]["out"].T
    return out.reshape(B, S, NX)


if __name__ == "__main__":
    rng = np.random.default_rng(0)
    hs = rng.standard_normal((B, S, NX), dtype=np.float32)
    wa = (rng.standard_normal((NX, 3 * NX), dtype=np.float32) * 0.02)
    wp = (rng.standard_normal((NX, NX), dtype=np.float32) * 0.02)
    o = kernel(hidden_states=hs, c_attn_w=wa, c_attn_b=np.zeros(3 * NX, np.float32),
               c_proj_w=wp, c_proj_b=np.zeros(NX, np.float32))
    print(o.shape, o.dtype)
